# revision 17
# baseline (speedup 1.0000x reference)
"""DeBERTa-style disentangled-attention BERT layer on 8 Trainium2 cores.

Data-parallel over batch (B=16 -> 2 per core). The position gather
pos[q,k] = t(q-k) is Toeplitz: we expand per-head tables
ekp[jj] = kp[t(511-jj)] via a one-hot matmul, compute W1r = Q @ ekp^T
windows on PE, stage them to DRAM, and read the gathered [q,k] matrix
back with an overlapping-stride DMA (addr = i*638 + 127 + k).
Attention runs in transposed layout ST[k, q]: masking and the softmax
denominator fold into an augmented-V matmul (ones column), so no
max-subtraction, no probs transpose, no divide pass over [512,512].
"""

import math
import sys

import numpy as np

sys.path.insert(0, "/opt/trn_rl_repo")

S = 512
B = 16
H = 768
NH = 12
HD = 64
NCORES = 8
BPC = B // NCORES  # batches per core
SCALE = 1.0 / math.sqrt(3 * HD)
EPS = 1e-7
EXPW = 1023
WINW = 639

_CACHE = {}


def _build_module():
    from contextlib import ExitStack

    import concourse.bass as bass
    import concourse.mybir as mybir
    import concourse.tile as tile
    from concourse import bacc
    from concourse.bass import AP

    fp32 = mybir.dt.float32
    bf16 = mybir.dt.bfloat16
    AX = mybir.AxisListType
    OP = mybir.AluOpType
    AF = mybir.ActivationFunctionType

    nc = bacc.Bacc("TRN2", target_bir_lowering=False)

    # ---- DRAM I/O ----
    hs_in = nc.dram_tensor("hs_in", [BPC, S, H], fp32, kind="ExternalInput")
    wqkT_in = nc.dram_tensor("wqkT_in", [769, 1536], bf16, kind="ExternalInput")
    wvT_in = nc.dram_tensor("wvT_in", [769, H], bf16, kind="ExternalInput")
    woT_in = nc.dram_tensor("woT_in", [769, H], bf16, kind="ExternalInput")
    relT_in = nc.dram_tensor("relT_in", [769, 63], bf16, kind="ExternalInput")
    mr1_in = nc.dram_tensor("mr1_in", [63, EXPW], bf16, kind="ExternalInput")
    mr2_in = nc.dram_tensor("mr2_in", [63, EXPW], bf16, kind="ExternalInput")
    nmask_in = nc.dram_tensor("nmask_in", [BPC, S], fp32, kind="ExternalInput")
    gb_in = nc.dram_tensor("gb_in", [128, 2 * H], fp32, kind="ExternalInput")
    idb_in = nc.dram_tensor("idb_in", [128, 128], bf16, kind="ExternalInput")
    idf_in = nc.dram_tensor("idf_in", [128, 128], fp32, kind="ExternalInput")

    out_o = nc.dram_tensor("out_o", [BPC, S, H], fp32, kind="ExternalOutput")
    ret_o = nc.dram_tensor("ret_o", [BPC, S, S], fp32, kind="ExternalOutput")

    CCH = [(c0, min(128, 769 - c0)) for c0 in range(0, 769, 128)]  # 6x128 + 1

    with ExitStack() as ctx:
        tc = ctx.enter_context(tile.TileContext(nc))
        # pools
        pc = ctx.enter_context(tc.tile_pool(name="const", bufs=1))
        pw = ctx.enter_context(tc.tile_pool(name="weights", bufs=1))
        ps = ctx.enter_context(tc.tile_pool(name="sbuf", bufs=2))
        pst = ctx.enter_context(tc.tile_pool(name="state", bufs=1))
        pA = ctx.enter_context(tc.tile_pool(name="psA", bufs=2, space="PSUM"))
        pB = ctx.enter_context(tc.tile_pool(name="psB", bufs=2, space="PSUM"))
        pC = ctx.enter_context(tc.tile_pool(name="psC", bufs=2, space="PSUM"))
        pD = ctx.enter_context(tc.tile_pool(name="dram", bufs=1, space="DRAM"))

        # ---- load constants / weights ----
        idb = pc.tile([128, 128], bf16, tag="idb")
        nc.sync.dma_start(idb[:, :], idb_in[:, :])
        idf = pc.tile([128, 128], fp32, tag="idf")
        nc.sync.dma_start(idf[:, :], idf_in[:, :])

        wqkT = pw.tile([128, 6 * 1536], bf16, tag="wqkT")  # rows 0..767
        for c in range(6):
            nc.sync.dma_start(wqkT[:, c * 1536:(c + 1) * 1536],
                              wqkT_in[c * 128:(c + 1) * 128, :])
        wqkB = pw.tile([1, 1536], bf16, tag="wqkB")  # row 768 = b_qk
        nc.sync.dma_start(wqkB[:, :], wqkT_in[768:769, :])

        wvT = pw.tile([128, 6 * H], bf16, tag="wvT")
        woT = pw.tile([128, 6 * H], bf16, tag="woT")
        for c in range(6):
            nc.sync.dma_start(wvT[:, c * H:(c + 1) * H], wvT_in[c * 128:(c + 1) * 128, :])
            nc.sync.dma_start(woT[:, c * H:(c + 1) * H], woT_in[c * 128:(c + 1) * 128, :])
        wvB = pw.tile([1, H], bf16, tag="wvB")
        nc.sync.dma_start(wvB[:, :], wvT_in[768:769, :])
        woB = pw.tile([1, H], bf16, tag="woB")
        nc.sync.dma_start(woB[:, :], woT_in[768:769, :])

        relTs = pw.tile([128, 6 * 63], bf16, tag="relTs")
        for c in range(6):
            nc.sync.dma_start(relTs[:, c * 63:(c + 1) * 63], relT_in[c * 128:(c + 1) * 128, :])
        relB = pw.tile([1, 63], bf16, tag="relB")
        nc.sync.dma_start(relB[:, :], relT_in[768:769, :])

        mr1 = pw.tile([63, EXPW], bf16, tag="mr1")
        nc.sync.dma_start(mr1[:, :], mr1_in[:, :])
        mr2 = pw.tile([63, EXPW], bf16, tag="mr2")
        nc.sync.dma_start(mr2[:, :], mr2_in[:, :])

        ones_row = pc.tile([1, 512], bf16, tag="ones_row")
        nc.gpsimd.memset(ones_row[:, :], 1.0)
        epsb = pc.tile([128, 1], fp32, tag="epsb")
        nc.gpsimd.memset(epsb[:, :], EPS)

        # gamma/beta broadcast [128, H] (host pre-broadcast)
        gbc = pc.tile([128, 2 * H], fp32, tag="gbc")
        nc.sync.dma_start(gbc[:, :], gb_in[:, :])
        ones12 = pc.tile([128, 12], bf16, tag="ones12")
        nc.gpsimd.memset(ones12[:, :], 1.0)

        # ---- rel projection: relP[p, h_out] = (relE @ w_qk.T + b_qk), bf16 ----
        relP = pst.tile([63, 1536], bf16, tag="relP")
        for n0 in range(0, 1536, 512):
            pr = pA.tile([63, 512], fp32, tag="pA")
            for ci, (c0, cn) in enumerate(CCH):
                if cn == 128:
                    lhs = relTs[:, (c0 // 128) * 63:(c0 // 128) * 63 + 63]
                    rhs = wqkT[:, (c0 // 128) * 1536 + n0:(c0 // 128) * 1536 + n0 + 512]
                else:
                    lhs = relB[:, :]
                    rhs = wqkB[:, n0:n0 + 512]
                nc.tensor.matmul(pr[:, :], lhs, rhs,
                                 start=(ci == 0), stop=(ci == len(CCH) - 1))
            nc.scalar.copy(relP[:, n0:n0 + 512], pr[:, :])

        # ---- per-batch state ----
        qkvT = [pst.tile([128, 12 * 512], bf16, tag=f"qkvT{b}", name=f"qkvT{b}") for b in range(BPC)]
        vba = [pst.tile([128, 4 * 780], bf16, tag=f"vba{b}", name=f"vba{b}") for b in range(BPC)]
        ctxs = [pst.tile([128, 4 * H], bf16, tag=f"ctxs{b}", name=f"ctxs{b}") for b in range(BPC)]
        reta = [pst.tile([128, 4 * 512], fp32, tag=f"reta{b}", name=f"reta{b}") for b in range(BPC)]

        w1d = [pD.tile([4, 128, WINW], bf16, tag=f"w1d{b}", name=f"w1d{b}") for b in range(BPC)]
        w2d = [pD.tile([4, 128, WINW], bf16, tag=f"w2d{b}", name=f"w2d{b}") for b in range(BPC)]

        def overlap_read(dtile, qt):
            # G[i, k] = W[i, 127 - i + k]  -> offset qt*(128*639) + 127, ap [[638,128],[1,512]]
            base = dtile[:, :, :]
            return AP(base.tensor, base.offset + qt * (128 * WINW) + 127,
                      [[WINW - 1, 128], [1, 512]])

        for b in range(BPC):
            # ---- LN + hsT ----
            hsT = ps.tile([128, 6 * 512], bf16, tag="hsT", bufs=1, name="hsT")
            for st in range(4):
                x = ps.tile([128, H], fp32, tag="lnx")
                nc.sync.dma_start(x[:, :], hs_in[b, st * 128:(st + 1) * 128, :])
                mu = ps.tile([128, 1], fp32, tag="lnmu")
                nc.vector.tensor_reduce(mu[:, :], x[:, :], AX.X, OP.add)
                nc.scalar.mul(mu[:, :], mu[:, :], 1.0 / H)
                ssum = ps.tile([128, 1], fp32, tag="lnssum")
                sq = ps.tile([128, H], fp32, tag="lnsq", bufs=1)
                nc.vector.tensor_tensor(sq[:, :], x[:, :], x[:, :], OP.mult)
                nc.vector.tensor_reduce(ssum[:, :], sq[:, :], AX.X, OP.add)
                nmm = ps.tile([128, 1], fp32, tag="lnnmm")
                nc.vector.tensor_tensor(nmm[:, :], mu[:, :], mu[:, :], OP.mult)
                nc.scalar.activation(nmm[:, :], nmm[:, :], AF.Copy,
                                     bias=EPS, scale=-1.0)
                std = ps.tile([128, 1], fp32, tag="lnstd")
                nc.scalar.activation(std[:, :], ssum[:, :], AF.Sqrt,
                                     bias=nmm[:, :], scale=1.0 / H)
                rinv = ps.tile([128, 1], fp32, tag="lnrinv")
                nc.vector.reciprocal(rinv[:, :], std[:, :])
                xcen = ps.tile([128, H], fp32, tag="lnxcen")
                nc.vector.tensor_scalar_sub(xcen[:, :], x[:, :], mu[:, :])
                xn = ps.tile([128, H], bf16, tag="lnxn")
                nc.vector.tensor_scalar_mul(xn[:, :], xcen[:, :], rinv[:, :])
                for hc in range(6):
                    tp = pC.tile([128, 128], bf16, tag="pC")
                    nc.tensor.transpose(tp[:, :], xn[:, hc * 128:(hc + 1) * 128], idb[:, :])
                    nc.scalar.copy(hsT[:, hc * 512 + st * 128:hc * 512 + (st + 1) * 128],
                                   tp[:, :])

            # ---- qkvT projection (scale folded into q rows) ----
            for mt in range(12):
                pq = pB.tile([128, 512], fp32, tag="pB")
                for ci, (c0, cn) in enumerate(CCH):
                    if cn == 128:
                        lhs = wqkT[:, (c0 // 128) * 1536 + mt * 128:(c0 // 128) * 1536 + (mt + 1) * 128]
                        rhs = hsT[:, (c0 // 128) * 512:(c0 // 128 + 1) * 512]
                    else:
                        lhs = wqkB[:, mt * 128:(mt + 1) * 128]
                        rhs = ones_row[:, :]
                    nc.tensor.matmul(pq[:, :], lhs, rhs,
                                     start=(ci == 0), stop=(ci == len(CCH) - 1))
                nc.scalar.mul(qkvT[b][:, mt * 512:(mt + 1) * 512], pq[:, :],
                              SCALE if mt < 6 else 1.0)

            # ---- v projection + augmented/masked V ----
            nm = ps.tile([128, 4], fp32, tag=f"nm{b}", name=f"nm{b}")
            nc.sync.dma_start(nm[:, :],
                              nmask_in[b, :].rearrange("(t p) -> p t", p=128))
            for st in range(4):
                pv = pA.tile([128, H], fp32, tag="pA")
                for n0 in range(0, H, 512):
                    nn = min(512, H - n0)
                    for ci, (c0, cn) in enumerate(CCH):
                        if cn == 128:
                            lhs = hsT[:, (c0 // 128) * 512 + st * 128:(c0 // 128) * 512 + (st + 1) * 128]
                            rhs = wvT[:, (c0 // 128) * H + n0:(c0 // 128) * H + n0 + nn]
                        else:
                            lhs = ones_row[:, st * 128:(st + 1) * 128]
                            rhs = wvB[:, n0:n0 + nn]
                        nc.tensor.matmul(pv[:, n0:n0 + nn], lhs, rhs,
                                         start=(ci == 0), stop=(ci == len(CCH) - 1))
                vdst = vba[b][:, st * 780:(st + 1) * 780].rearrange(
                    "p (h d) -> p h d", h=12)
                nc.vector.tensor_scalar_mul(
                    vdst[:, :, 0:64],
                    pv[:, :].rearrange("p (h d) -> p h d", h=12), nm[:, st:st + 1])
                nmrep = ps.tile([128, 12], bf16, tag="nmrep")
                nc.vector.tensor_scalar_mul(nmrep[:, :], ones12[:, :], nm[:, st:st + 1])
                nc.vector.tensor_copy(vdst[:, :, 64:65], nmrep[:, :].unsqueeze(2))

            nc.gpsimd.memset(reta[b][:, :], 0.0)

        # ---- per head: tables, expansion windows, attention ----
        for h in range(12):
            mtq, po = h // 2, 64 * (h % 2)
            mtk = 6 + h // 2
            kp_h = relP[:, H + 64 * h:H + 64 * (h + 1)]
            qp_h = relP[:, 64 * h:64 * (h + 1)]

            ekpT = ps.tile([128, EXPW], bf16, tag="ekpT")
            eqpT = ps.tile([128, EXPW], bf16, tag="eqpT")
            for (dst, srcv, mr, sc) in ((ekpT, kp_h, mr1, 1.0),
                                        (eqpT, qp_h, mr2, SCALE)):
                pe = pA.tile([128, EXPW], fp32, tag="pA")
                for p0 in (0, 64):
                    for n0 in (0, 512):
                        nn = min(512, EXPW - n0)
                        nc.tensor.matmul(pe[p0:p0 + 64, n0:n0 + nn], srcv,
                                         mr[:, n0:n0 + nn], start=True, stop=True)
                nc.scalar.mul(dst[:, :], pe[:, :], sc)

            for b in range(BPC):
                qT = qkvT[b][po:po + 64, mtq * 512:(mtq + 1) * 512]
                kT = qkvT[b][po:po + 64, mtk * 512:(mtk + 1) * 512]

                # W expansion windows -> DRAM
                for (lo, tab, wdst) in ((qT, ekpT, w1d[b]), (kT, eqpT, w2d[b])):
                    for t in range(4):
                        a = 384 - t * 128
                        wp = pA.tile([128, WINW], fp32, tag="pA")
                        nc.tensor.matmul(wp[:, 0:512],
                                         lo[:, t * 128:(t + 1) * 128],
                                         tab[po:po + 64, a:a + 512],
                                         start=True, stop=True)
                        nc.tensor.matmul(wp[:, 512:WINW],
                                         lo[:, t * 128:(t + 1) * 128],
                                         tab[po:po + 64, a + 512:a + WINW],
                                         start=True, stop=True)
                        wsb = ps.tile([128, WINW], bf16, tag="wsb")
                        nc.scalar.copy(wsb[:, :], wp[:, :])
                        nc.sync.dma_start(wdst[t, :, :], wsb[:, :])

                # gather reads
                g1 = [ps.tile([128, 512], bf16, tag=f"g1_{qt}", bufs=1, name=f"g1_{qt}")
                      for qt in range(4)]
                for qt in range(4):
                    nc.sync.dma_start(g1[qt][:, :], overlap_read(w1d[b], qt))

                expst = [ps.tile([128, 512], bf16, tag=f"expst{kt}", bufs=1, name=f"expst{kt}")
                         for kt in range(4)]
                for kt in range(4):
                    stp = pB.tile([128, 512], fp32, tag="pB")
                    nc.tensor.matmul(stp[:, :], kT[:, kt * 128:(kt + 1) * 128], qT,
                                     start=True, stop=False)
                    for qt in range(4):
                        nc.tensor.matmul(
                            stp[:, qt * 128:(qt + 1) * 128],
                            g1[qt][:, kt * 128:(kt + 1) * 128], idb[:, :],
                            start=False, stop=(qt == 3), skip_group_check=True)
                    g2 = ps.tile([128, 512], bf16, tag="g2")
                    nc.sync.dma_start(g2[:, :], overlap_read(w2d[b], kt))
                    nc.vector.tensor_tensor(stp[:, :], stp[:, :], g2[:, :], OP.add)
                    nc.vector.tensor_tensor(reta[b][:, kt * 512:(kt + 1) * 512],
                                            reta[b][:, kt * 512:(kt + 1) * 512],
                                            stp[:, :], OP.add)
                    nc.scalar.activation(expst[kt][:, :], stp[:, :], AF.Exp)

                for qt in range(4):
                    cp = pC.tile([128, 128], fp32, tag="pC")
                    for kt in range(4):
                        nc.tensor.matmul(cp[:, 0:65],
                                         expst[kt][:, qt * 128:(qt + 1) * 128],
                                         vba[b][:, kt * 780 + 65 * h:kt * 780 + 65 * (h + 1)],
                                         start=(kt == 0), stop=(kt == 3))
                    rz = ps.tile([128, 1], fp32, tag="rz")
                    nc.vector.reciprocal(rz[:, :], cp[:, 64:65])
                    nc.vector.tensor_scalar_mul(
                        ctxs[b][:, qt * H + 64 * h:qt * H + 64 * (h + 1)],
                        cp[:, 0:64], rz[:, :])

        # ---- output projection + post-LN + returned scores ----
        for b in range(BPC):
            ctxT = ps.tile([128, 6 * 512], bf16, tag="ctxT", bufs=1)
            for st in range(4):
                for hc in range(6):
                    tp = pC.tile([128, 128], bf16, tag="pC")
                    nc.tensor.transpose(tp[:, :],
                                        ctxs[b][:, st * H + hc * 128:st * H + (hc + 1) * 128],
                                        idb[:, :])
                    nc.scalar.copy(ctxT[:, hc * 512 + st * 128:hc * 512 + (st + 1) * 128],
                                   tp[:, :])
            for st in range(4):
                po_ = pA.tile([128, H], fp32, tag="pA")
                for n0 in range(0, H, 512):
                    nn = min(512, H - n0)
                    for hc in range(6):
                        nc.tensor.matmul(po_[:, n0:n0 + nn],
                                         ctxT[:, hc * 512 + st * 128:hc * 512 + (st + 1) * 128],
                                         woT[:, hc * H + n0:hc * H + n0 + nn],
                                         start=(hc == 0), stop=False)
                    nc.tensor.matmul(po_[:, n0:n0 + nn],
                                     ones_row[:, st * 128:(st + 1) * 128],
                                     woB[:, n0:n0 + nn], start=False, stop=True)
                mu = ps.tile([128, 1], fp32, tag="lnmu")
                nc.vector.tensor_reduce(mu[:, :], po_[:, :], AX.X, OP.add)
                nc.scalar.mul(mu[:, :], mu[:, :], 1.0 / H)
                ssum = ps.tile([128, 1], fp32, tag="lnssum")
                sq = ps.tile([128, H], fp32, tag="lnsq", bufs=1)
                nc.scalar.activation(sq[:, :], po_[:, :], AF.Square)
                nc.vector.tensor_reduce(ssum[:, :], sq[:, :], AX.X, OP.add)
                nmm = ps.tile([128, 1], fp32, tag="lnnmm")
                nc.vector.tensor_tensor(nmm[:, :], mu[:, :], mu[:, :], OP.mult)
                nc.scalar.activation(nmm[:, :], nmm[:, :], AF.Copy,
                                     bias=EPS, scale=-1.0)
                std = ps.tile([128, 1], fp32, tag="lnstd")
                nc.scalar.activation(std[:, :], ssum[:, :], AF.Sqrt,
                                     bias=nmm[:, :], scale=1.0 / H)
                rinv = ps.tile([128, 1], fp32, tag="lnrinv")
                nc.vector.reciprocal(rinv[:, :], std[:, :])
                xg = ps.tile([128, H], fp32, tag="xg")
                nc.vector.tensor_scalar_sub(xg[:, :], po_[:, :], mu[:, :])
                nc.vector.tensor_scalar_mul(xg[:, :], xg[:, :], rinv[:, :])
                nc.vector.tensor_tensor(xg[:, :], xg[:, :], gbc[:, 0:H], OP.mult)
                nc.vector.tensor_tensor(xg[:, :], xg[:, :], gbc[:, H:2 * H], OP.add)
                nc.sync.dma_start(out_o[b, st * 128:(st + 1) * 128, :], xg[:, :])

            for qt in range(4):
                rq = ps.tile([128, 512], fp32, tag="rq", bufs=1)
                for kt in range(4):
                    tp = pC.tile([128, 128], fp32, tag="pC")
                    nc.tensor.transpose(
                        tp[:, :],
                        reta[b][:, kt * 512 + qt * 128:kt * 512 + (qt + 1) * 128],
                        idf[:, :])
                    nc.scalar.mul(rq[:, kt * 128:(kt + 1) * 128], tp[:, :], 1.0 / 12.0)
                nc.sync.dma_start(ret_o[b, qt * 128:(qt + 1) * 128, :], rq[:, :])

    nc.compile()
    return nc


def _host_prep(hidden_states, relative_embedding, w_qk, b_qk, w_v, b_v,
               w_o, b_o, ln_gamma, ln_beta, attention_mask, position_indices):
    bf = np.dtype("bfloat16") if hasattr(np, "bfloat16") else None
    import ml_dtypes
    bf = ml_dtypes.bfloat16

    wqkT = np.concatenate([w_qk.T, b_qk[None, :]], 0).astype(bf)          # [769,1536]
    wvT = np.concatenate([w_v.T, b_v[None, :]], 0).astype(bf)             # [769,768]
    woT = np.concatenate([w_o.T, b_o[None, :]], 0).astype(bf)             # [769,768]
    relT = np.concatenate([relative_embedding.T,
                           np.ones((1, 63), np.float32)], 0).astype(bf)   # [769,63]

    pos = np.asarray(position_indices)
    t_arr = np.empty(EXPW, np.int64)
    t_arr[511:] = pos[511, 511 - np.arange(512)]     # d >= 0
    t_arr[:511] = pos[0, 511 - np.arange(511)]       # d < 0
    jj = np.arange(EXPW)
    mr2 = (t_arr[None, jj] == np.arange(63)[:, None]).astype(np.float32).astype(bf)
    mr1 = (t_arr[None, 1022 - jj] == np.arange(63)[:, None]).astype(np.float32).astype(bf)

    nmask = (1.0 - np.asarray(attention_mask)
             .reshape(B, S).astype(np.float32))                           # [B,S]
    gb = np.tile(np.concatenate([np.asarray(ln_gamma), np.asarray(ln_beta)])[None, :],
                 (128, 1)).astype(np.float32)
    idb = np.eye(128, dtype=np.float32).astype(bf)
    idf = np.eye(128, dtype=np.float32)

    hs = np.asarray(hidden_states, np.float32)
    in_maps = []
    for c in range(NCORES):
        in_maps.append({
            "hs_in": np.ascontiguousarray(hs[:, c * BPC:(c + 1) * BPC, :].transpose(1, 0, 2)),
            "wqkT_in": wqkT, "wvT_in": wvT, "woT_in": woT, "relT_in": relT,
            "mr1_in": mr1, "mr2_in": mr2,
            "nmask_in": np.ascontiguousarray(nmask[c * BPC:(c + 1) * BPC]),
            "gb_in": gb, "idb_in": idb, "idf_in": idf,
        })
    return in_maps


def kernel(**inputs):
    from concourse import bass_utils

    if "nc" not in _CACHE:
        _CACHE["nc"] = _build_module()
    nc = _CACHE["nc"]
    in_maps = _host_prep(**inputs)
    res = bass_utils.run_bass_kernel_spmd(nc, in_maps, core_ids=list(range(NCORES)))
    outs = res.results
    out = np.concatenate([outs[c]["out_o"].transpose(1, 0, 2)
                          for c in range(NCORES)], axis=1)
    ret = np.concatenate([outs[c]["ret_o"] for c in range(NCORES)], axis=0)
    return out.astype(np.float32), ret.astype(np.float32)
